# revision 2
# baseline (speedup 1.0000x reference)
"""Trainium2 Bass kernel for nn_CoarseMatching (dual-softmax coarse matching).

Computes, for x0/x1 of shape [2, 6400, 256]:
    sim   = x0 @ x1^T / (C * temperature)                       [n, l, s]
    conf  = softmax(sim, axis=2) * softmax(sim, axis=1)
          = exp(2*sim - log(rowsum) - log(colsum))
    mask  = (conf > 0.2) & border_valid & mutual-argmax(conf)
    scores= where(mask, conf, 0)

Distribution: the l (query) axis is sharded over 8 NeuronCores (800 rows
per core, both batches). Per core (default mode "exp2"):
  P1: sim via bf16 matmuls (fp32 PSUM accum, x0 pre-scaled by
      2/(C*temperature)); one ACT pass E = exp(sim) whose accum_out
      yields per-row sums and whose bf16 output feeds a ones-matmul on
      the tensor engine for partial per-column sums. Column sums are
      AllReduce'd across the 8 cores in-kernel.
  P2: recompute 2*sim (2048-wide PSUM chunks), one ACT pass
      u = exp(2*sim - log rowsum - 6 ln2) using the per-partition bias
      slot, then a 2x-mode DVE tensor_tensor multiply against a
      broadcast fp16 column-scale tile cs = 2^6/colsum (built from the
      AllReduce result with ACT ln/exp and a tensor-engine ones
      broadcast). conf is written in bf16 and DMA'd out.

mask/scores are exactly zero whenever no conf comes near the 0.2
threshold (checked on host from the returned conf; a numpy fallback
reproduces the exact reference semantics otherwise).

Execution goes through a cached PJRT runner (jit + shard_map over the 8
axon NeuronCores) that keeps inputs and the output placeholders
device-resident, so repeated invocations move no host data.

Alternative modes kept for diagnostics: "exp" (log-colsum rides two
bf16 contraction rows into the P2 matmul; ACT-only normalization),
"dve" (E staged to DRAM, conf = E^2/(rowsum*colsum) on the DVE), and
"exp3" (per-batch AllReduce). Build flags no_ar/do_p1/do_p2/do_dma
exist for timing decomposition only.
"""

import os
import sys

import numpy as np

# The Bass kernel executes on the axon-tunneled NeuronCores via PJRT; make
# sure the axon platform stays available even if the caller pinned
# JAX_PLATFORMS=cpu (keep cpu first so the caller's default backend is
# unchanged).
if "jax" not in sys.modules:
    _jp = os.environ.get("JAX_PLATFORMS")
    if _jp and "axon" not in _jp.split(","):
        os.environ["JAX_PLATFORMS"] = _jp + ",axon"

# ---------------------------------------------------------------------------
# BIR post-pass: split instructions with >1 sync wait into single-wait chains.
# The TRN2 ISA carries one wait slot per instruction; this walrus build
# refuses multi-wait BIR instructions instead of splitting them. Splitting is
# semantics-preserving (waits gate dispatch on the engine's serial stream).
# ---------------------------------------------------------------------------
import orjson

_counter = [0]


def _split_bir(bir_json: bytes) -> bytes:
    bir = orjson.loads(bir_json)
    changed = False
    for fn in bir.get("functions", []):
        for bb in fn.get("blocks", []):
            insts = bb.get("instructions", [])
            out = []
            for inst in insts:
                si = inst.get("sync_info")
                waits = (si or {}).get("on_wait") or []
                keep = 0 if inst.get("opcode") == "Matmult" else 1
                if len(waits) > keep:
                    changed = True
                    for w in waits[: len(waits) - keep]:
                        _counter[0] += 1
                        out.append({
                            "debug": inst.get("debug", 0),
                            "engine": inst["engine"],
                            "ins": [],
                            "name": f"splitwait-{_counter[0]}-{inst['name']}",
                            "opcode": "EventSemaphore",
                            "outs": [],
                            "sync_info": {"on_update": [], "on_wait": [w]},
                        })
                    si["on_wait"] = waits[len(waits) - keep:]
                out.append(inst)
            bb["instructions"] = out
    if not changed:
        return bir_json
    return orjson.dumps(bir)


_installed = [False]


def _install_bir_fix():
    if _installed[0]:
        return
    _installed[0] = True
    import concourse.bass_utils as bu
    import concourse.bass2jax as b2j

    orig = bu.compile_bir_kernel

    def patched(bir_json, tmpdir, neff_name="file.neff"):
        return orig(_split_bir(bir_json), tmpdir, neff_name=neff_name)

    bu.compile_bir_kernel = patched
    b2j.compile_bir_kernel = patched


# ---------------------------------------------------------------------------
# Problem constants (hardcoded per spec)
# ---------------------------------------------------------------------------
N, L, S, C = 2, 6400, 6400, 256
THRESHOLD = 0.2
BORDER = 2
TEMPERATURE = 0.1
H0 = W0 = H1 = W1 = 80
NCORES = 8
LSH = L // NCORES                      # 800 rows per core
SCALE2 = 2.0 / (C * TEMPERATURE)       # x0 pre-scale so matmul yields 2*sim

GROUPS = [(g * 1024, 1024) for g in range(6)] + [(6144, 256)]  # 7 ACT groups
STRIPS = [(k * 128, 128) for k in range(6)] + [(768, 32)]      # 7 per batch
NSTRIP = len(STRIPS) * N                                       # 14
NGRP = len(GROUPS)                                             # 7


def _halves(gw, wide=False):
    if wide:
        return [(0, gw)]
    return [(0, 512), (512, 512)] if gw == 1024 else [(0, gw)]


def build_kernel(reps=1, do_p1=True, do_p2=True, do_dma=True, mode="exp2",
                 no_ar=False, wide=False, p2w=2048):
    """mode="exp": P2 recomputes 2*sim on PE and applies exp on ACT (log
    colsum rides two bf16 contraction rows, log rowsum the ACT bias).
    mode="dve": P1 stages E=exp(sim) to DRAM; P2 reloads E and computes
    conf = E^2 * (1/rowsum) * (1/colsum) with two DVE passes (per-
    partition scalar = 1/rowsum, broadcast tile = 1/colsum)."""
    import concourse.bass as bass
    import concourse.mybir as mybir
    import concourse.tile as tile

    F32 = mybir.dt.float32
    BF16 = mybir.dt.bfloat16
    FP16 = mybir.dt.float16
    # E-path dtype: fp16 in dve mode (E gets squared, so its rounding error
    # doubles; fp16 halves mantissa error 8x and exp(sim) \in [0.04, 25] sits
    # safely in fp16's normal range). bf16 suffices for the colsum-only use.
    EDT = FP16 if mode == "dve" else BF16
    AF = mybir.ActivationFunctionType
    MUL = mybir.AluOpType.mult

    nc = bass.Bass(trn_type="TRN2", target_bir_lowering=False, debug=False,
                   num_devices=NCORES)

    x0t = nc.dram_tensor("x0t", [N, C, LSH], BF16, kind="ExternalInput")
    x1t = nc.dram_tensor("x1t", [N, C, S], BF16, kind="ExternalInput")
    conf_d = nc.dram_tensor("conf", [N, LSH, S], BF16, kind="ExternalOutput")
    rowstat_d = nc.dram_tensor("rowstat", [128, NSTRIP], F32, kind="ExternalOutput")
    E_d = None
    if mode == "dve":
        E_d = nc.dram_tensor("E_d", [N, LSH, S], FP16, kind="Internal")

    with tile.TileContext(nc) as tc:
        with tc.tile_pool(name="persist", bufs=1) as pp, \
             tc.tile_pool(name="epool", bufs=4) as ep, \
             tc.tile_pool(name="eload", bufs=4) as lp, \
             tc.tile_pool(name="tmppool", bufs=3) as tp, \
             tc.tile_pool(name="confpool", bufs=6) as cp, \
             tc.tile_pool(name="dram", bufs=1, space="DRAM") as dp:

            # ---- persistent tiles -------------------------------------
            x1s = [[pp.tile([128, S], BF16, tag=f"x1_{n}_{kb}", name=f"x1_{n}_{kb}")
                    for kb in range(2)] for n in range(N)]
            x0s = [[pp.tile([128, LSH], BF16, tag=f"x0_{n}_{kb}", name=f"x0_{n}_{kb}")
                    for kb in range(2)] for n in range(N)]
            ones_col = pp.tile([128, 128], EDT, tag="ones_col")
            neg_ones = pp.tile([2, 128], BF16, tag="neg_ones")
            # column stats live at partitions {0,...} x free groups:
            # cidx = n*NGRP+g -> partition 32*(cidx%NP_), free grp cidx//NP_.
            # exp keeps the 4-partition layout; dve/exp2 use 3 partitions so
            # the PE broadcast matmuls stay at base partition <= 64.
            NP_ = 4 if mode == "exp" else 3
            # free groups: exp 4 (14 cells / 4 partitions); exp2/dve 5
            # (14/3); exp3 3 per batch * 2 batches (7/3 each)
            NGB_ = -(-NGRP // 3)                      # 3, per-batch (exp3)
            NG_ = N * NGB_ if mode == "exp3" else -(-(N * NGRP) // NP_)
            colsum4 = pp.tile([128, NG_ * 1024], F32, tag="colsum4")
            lc4 = pp.tile([128, NG_ * 1024], F32, tag="lc4")
            if mode == "exp":
                # bf16 hi/lo of log(colsum) reuse colsum4's dead bytes
                hi4 = colsum4[:].bitcast(BF16)[:, 0:4 * 1024]
                lo4 = colsum4[:].bitcast(BF16)[:, 4 * 1024:8 * 1024]
                aug = pp.tile([2, NSTRIP * 1024], BF16, tag="aug")
            rowsum_parts = pp.tile([128, NSTRIP * NGRP], F32, tag="rsp")
            confsum_parts = pp.tile([128, NSTRIP * NGRP], F32, tag="csp")
            rowsum_tot = pp.tile([128, NSTRIP], F32, tag="rst")
            neg_lr = pp.tile([128, NSTRIP], F32, tag="nlr")
            confsum_tot = pp.tile([128, NSTRIP], F32, tag="cst")
            if mode == "dve":
                rs_t = pp.tile([128, NSTRIP], F32, tag="rs")      # 1/rowsum
            if mode in ("dve", "exp2", "exp3"):
                csb_src = pp.tile([128, NG_ * 1024], FP16, tag="csbs")
                cs_b = pp.tile([128, N * NGRP * 1024], FP16, tag="csb")
                ones_bc = pp.tile([128, 128], FP16, tag="ones_bc")
                bln6 = pp.tile([128, 1], F32, tag="bln6")   # 6 ln2
                bln12 = pp.tile([128, 1], F32, tag="bln12")  # 12 ln2

            if mode == "exp3":
                cc_inB = [dp.tile([3, NGB_ * 1024], F32, tag=f"cc_in{n}",
                                  name=f"cc_in{n}") for n in range(N)]
                cc_outB = [dp.tile([3, NGB_ * 1024], F32, tag=f"cc_out{n}",
                                   name=f"cc_out{n}") for n in range(N)]
            else:
                cc_in = dp.tile([NP_, NG_ * 1024], F32, tag="cc_in")
                cc_out = dp.tile([NP_, NG_ * 1024], F32, tag="cc_out")

            # ---- loads + consts ---------------------------------------
            for n in range(N):
                for kb in range(2):
                    nc.sync.dma_start(x1s[n][kb][:], x1t[n, kb * 128:(kb + 1) * 128, :])
                    nc.sync.dma_start(x0s[n][kb][:], x0t[n, kb * 128:(kb + 1) * 128, :])
            nc.gpsimd.memset(ones_col[:], 1.0)
            nc.gpsimd.memset(neg_ones[:], -1.0)
            if mode in ("dve", "exp2", "exp3"):
                nc.gpsimd.memset(ones_bc[:], 1.0)
                nc.vector.memset(bln6[:], 6.0 * 0.6931471805599453)
                nc.vector.memset(bln12[:], 12.0 * 0.6931471805599453)
            nc.vector.memset(confsum_parts[:], 0.0)
            nc.vector.memset(rowsum_parts[:], 1e-30)
            nc.vector.memset(colsum4[:], 1.0)
            if mode in ("dve", "exp2", "exp3"):
                nc.vector.memset(lc4[:], 1.0)

            for _rep in range(reps):
                if not do_p1:
                    nc.vector.memset(aug[:], 0.0)
                    nc.vector.memset(neg_lr[:], -18.0)
                if not do_p2:
                    nc.vector.memset(confsum_tot[:], 0.0)

                # =========================================================
                # Phase 1: stats (rowsum via ACT accum, colsum via PE ones)
                # =========================================================
                if do_p1 and mode != "exp3":
                    with tc.tile_pool(name="ps1", bufs=3, space="PSUM") as ps1, \
                         tc.tile_pool(name="pc1", bufs=1, space="PSUM") as pc1:
                        for n in range(N):
                            for g, (c0, gw) in enumerate(GROUPS):
                                pcol = pc1.tile([128, 1024], F32, tag="pcol")
                                for i, (l0, rows) in enumerate(STRIPS):
                                    sidx = n * len(STRIPS) + i
                                    psim = ps1.tile([128, 1024], F32, tag="psim")
                                    for h0, hw in _halves(gw, wide):
                                        for kb in range(2):
                                            nc.tensor.matmul(
                                                psim[:rows, h0:h0 + hw],
                                                x0s[n][kb][:, l0:l0 + rows],
                                                x1s[n][kb][:, c0 + h0:c0 + h0 + hw],
                                                start=(kb == 0), stop=(kb == 1))
                                    e = ep.tile([128, 1024], EDT, tag="e")
                                    nc.scalar.activation(
                                        e[:rows, :gw], psim[:rows, :gw], AF.Exp,
                                        scale=0.5,
                                        accum_out=rowsum_parts[:rows,
                                                               sidx * NGRP + g:
                                                               sidx * NGRP + g + 1])
                                    for h0, hw in _halves(gw, wide):
                                        nc.tensor.matmul(
                                            pcol[:, h0:h0 + hw],
                                            ones_col[:rows, :],
                                            e[:rows, h0:h0 + hw],
                                            start=(i == 0),
                                            stop=(i == len(STRIPS) - 1))
                                    if mode == "dve":
                                        nc.gpsimd.dma_start(
                                            E_d[n, l0:l0 + rows, c0:c0 + gw],
                                            e[:rows, :gw])
                                cidx = n * NGRP + g
                                cp_ = 32 * (cidx % NP_)
                                cg_ = (cidx // NP_) * 1024
                                nc.vector.tensor_copy(
                                    colsum4[cp_:cp_ + 1, cg_:cg_ + gw],
                                    pcol[0:1, :gw])

                    # row stats per strip
                    nc.vector.reduce_sum(
                        rowsum_tot[:],
                        rowsum_parts[:].rearrange("p (s j) -> p s j", j=NGRP),
                        axis=mybir.AxisListType.X)
                    if mode in ("exp", "exp2"):
                        nc.scalar.activation(neg_lr[:], rowsum_tot[:], AF.Ln)
                        nc.vector.tensor_scalar_mul(neg_lr[:], neg_lr[:], -1.0)
                        if mode == "exp2":
                            # absorb cs's 2^6 scale: exp(.. - lr - 6 ln2)
                            nc.vector.tensor_scalar_add(
                                neg_lr[:], neg_lr[:],
                                -6.0 * 0.6931471805599453)
                    else:
                        # rs = 2^12/rowsum via exp(-ln(rowsum) + 12 ln2); the
                        # 2^12 (with cs's 2^6) keeps the fp16 intermediates
                        # normal; the P2 instr2 un-scales by 2^-18.
                        nc.scalar.activation(neg_lr[:], rowsum_tot[:], AF.Ln)
                        nc.scalar.activation(
                            rs_t[:], neg_lr[:], AF.Exp, scale=-1.0,
                            bias=bln12[:, 0:1])

                    # column stats: AllReduce over the 8 cores
                    nc.gpsimd.dma_start(cc_in[:], colsum4[0:NP_ * 32:32, :])
                    if no_ar:
                        # timing-diagnostic build: local copy instead of the
                        # collective (results numerically wrong by 8x)
                        nc.gpsimd.dma_start(cc_out[:], cc_in[:])
                    else:
                        nc.gpsimd.collective_compute(
                            "AllReduce", mybir.AluOpType.add,
                            ins=[cc_in[:]], outs=[cc_out[:]],
                            replica_groups=[list(range(NCORES))])
                    nc.gpsimd.dma_start(lc4[0:NP_ * 32:32, :], cc_out[:])
                    if mode in ("dve", "exp2"):
                        # cs = 2^6/colsum via exp(-ln(colsum) + 6 ln2), fp16,
                        # then broadcast each (n,g) cell across all 128
                        # partitions for the P2 column-scale multiply. The
                        # broadcast rides the tensor engine: ones[1,128]^T @
                        # cs_cell[1,512] replicates a row into all 128 PSUM
                        # partitions; DVE copies to SBUF.
                        for p3 in range(NP_):
                            r = lc4[32 * p3:32 * p3 + 1, :]
                            nc.scalar.activation(r, r, AF.Ln)
                            nc.scalar.activation(
                                csb_src[32 * p3:32 * p3 + 1, :], r, AF.Exp,
                                scale=-1.0,
                                bias=bln6[32 * p3:32 * p3 + 1, 0:1])
                        with tc.tile_pool(name="psb", bufs=2,
                                          space="PSUM") as psb:
                            for n in range(N):
                                for g in range(NGRP):
                                    cidx = n * NGRP + g
                                    cp_, cg_ = (32 * (cidx % NP_),
                                                (cidx // NP_) * 1024)
                                    pb = psb.tile([128, 1024], F32, tag="pb")
                                    for h0 in (0, 512):
                                        nc.tensor.matmul(
                                            pb[:, h0:h0 + 512],
                                            ones_bc[cp_:cp_ + 1, :],
                                            csb_src[cp_:cp_ + 1,
                                                    cg_ + h0:cg_ + h0 + 512],
                                            start=True, stop=True)
                                    nc.vector.tensor_copy(
                                        cs_b[:, cidx * 1024:cidx * 1024 + 1024],
                                        pb[:])
                    if mode == "exp":
                        for p4 in range(4):
                            nc.scalar.activation(lc4[32 * p4:32 * p4 + 1, :],
                                                 lc4[32 * p4:32 * p4 + 1, :], AF.Ln)
                        # split log(colsum) into bf16 hi+lo (scratch: colsum4's
                        # bytes, now dead), then scatter into `aug` rows via
                        # strided DMAs: aug offset cidx*1024, cidx = g4*4 + p4,
                        # cell at partition 32*p4, free group g4.
                        nc.vector.tensor_copy(hi4, lc4[:])
                        nc.vector.scalar_tensor_tensor(
                            lo4, lc4[:], 1.0, hi4,
                            op0=mybir.AluOpType.mult, op1=mybir.AluOpType.subtract)
                        for p4 in range(4):
                            ng = 4 if p4 < 2 else 3
                            for row, srct in ((0, hi4), (1, lo4)):
                                dst = aug[row:row + 1,
                                          p4 * 1024:
                                          p4 * 1024 + (ng - 1) * 4096 + 1024]
                                nc.sync.dma_start(
                                    dst.rearrange("o (g t) -> o g t", t=1024)[:, ::4, :],
                                    srct[32 * p4:32 * p4 + 1, 0:ng * 1024]
                                    .rearrange("o (g t) -> o g t", t=1024))

                # =========================================================
                # exp3: per-batch P1 + AllReduce, so each batch's collective
                # hides behind the other batch's compute.
                # =========================================================
                if mode == "exp3":
                    for n in range(N):
                        with tc.tile_pool(name="ps1", bufs=3,
                                          space="PSUM") as ps1, \
                             tc.tile_pool(name="pc1", bufs=1,
                                          space="PSUM") as pc1:
                            for g, (c0, gw) in enumerate(GROUPS):
                                pcol = pc1.tile([128, 1024], F32, tag="pcol")
                                for i, (l0, rows) in enumerate(STRIPS):
                                    sidx = n * len(STRIPS) + i
                                    psim = ps1.tile([128, 1024], F32,
                                                    tag="psim")
                                    for h0, hw in _halves(gw, wide):
                                        for kb in range(2):
                                            nc.tensor.matmul(
                                                psim[:rows, h0:h0 + hw],
                                                x0s[n][kb][:, l0:l0 + rows],
                                                x1s[n][kb][:,
                                                           c0 + h0:
                                                           c0 + h0 + hw],
                                                start=(kb == 0),
                                                stop=(kb == 1))
                                    e = ep.tile([128, 1024], EDT, tag="e")
                                    nc.scalar.activation(
                                        e[:rows, :gw], psim[:rows, :gw],
                                        AF.Exp, scale=0.5,
                                        accum_out=rowsum_parts[
                                            :rows, sidx * NGRP + g:
                                            sidx * NGRP + g + 1])
                                    for h0, hw in _halves(gw, wide):
                                        nc.tensor.matmul(
                                            pcol[:, h0:h0 + hw],
                                            ones_col[:rows, :],
                                            e[:rows, h0:h0 + hw],
                                            start=(i == 0),
                                            stop=(i == len(STRIPS) - 1))
                                cpp = 32 * (g % 3)
                                cgg = (n * NGB_ + g // 3) * 1024
                                nc.vector.tensor_copy(
                                    colsum4[cpp:cpp + 1, cgg:cgg + gw],
                                    pcol[0:1, :gw])

                        # batch-n row stats: -log(rowsum) - 6 ln2
                        s7 = len(STRIPS)
                        nlr = neg_lr[:, n * s7:(n + 1) * s7]
                        nc.vector.reduce_sum(
                            nlr,
                            rowsum_parts[:, n * s7 * NGRP:(n + 1) * s7 * NGRP]
                            .rearrange("p (s j) -> p s j", j=NGRP),
                            axis=mybir.AxisListType.X)
                        nc.scalar.activation(nlr, nlr, AF.Ln)
                        nc.vector.tensor_scalar_mul(nlr, nlr, -1.0)
                        nc.vector.tensor_scalar_add(
                            nlr, nlr, -6.0 * 0.6931471805599453)

                        # batch-n column stats: AllReduce + cs broadcast
                        cofs = n * NGB_ * 1024
                        nc.gpsimd.dma_start(
                            cc_inB[n][:],
                            colsum4[0:96:32, cofs:cofs + NGB_ * 1024])
                        if no_ar:
                            nc.gpsimd.dma_start(cc_outB[n][:], cc_inB[n][:])
                        else:
                            nc.gpsimd.collective_compute(
                                "AllReduce", mybir.AluOpType.add,
                                ins=[cc_inB[n][:]], outs=[cc_outB[n][:]],
                                replica_groups=[list(range(NCORES))])
                        nc.gpsimd.dma_start(
                            lc4[0:96:32, cofs:cofs + NGB_ * 1024],
                            cc_outB[n][:])
                        for p3 in range(3):
                            r = lc4[32 * p3:32 * p3 + 1,
                                    cofs:cofs + NGB_ * 1024]
                            nc.scalar.activation(r, r, AF.Ln)
                            nc.scalar.activation(
                                csb_src[32 * p3:32 * p3 + 1,
                                        cofs:cofs + NGB_ * 1024],
                                r, AF.Exp, scale=-1.0,
                                bias=bln6[32 * p3:32 * p3 + 1, 0:1])
                        with tc.tile_pool(name="psb", bufs=2,
                                          space="PSUM") as psb:
                            for g in range(NGRP):
                                cpp = 32 * (g % 3)
                                cgg = (n * NGB_ + g // 3) * 1024
                                cidx = n * NGRP + g
                                pb = psb.tile([128, 1024], F32, tag="pb")
                                for h0 in (0, 512):
                                    nc.tensor.matmul(
                                        pb[:, h0:h0 + 512],
                                        ones_bc[cpp:cpp + 1, :],
                                        csb_src[cpp:cpp + 1,
                                                cgg + h0:cgg + h0 + 512],
                                        start=True, stop=True)
                                nc.vector.tensor_copy(
                                    cs_b[:, cidx * 1024:cidx * 1024 + 1024],
                                    pb[:])

                # =========================================================
                # Phase 2: conf = exp(2*sim - log colsum - log rowsum)
                # =========================================================
                if do_p2 and mode == "exp":
                    with tc.tile_pool(name="ps2", bufs=4, space="PSUM") as ps2:
                        for n in range(N):
                            for i, (l0, rows) in enumerate(STRIPS):
                                sidx = n * len(STRIPS) + i
                                for g, (c0, gw) in enumerate(GROUPS):
                                    cidx = n * NGRP + g
                                    psim = ps2.tile([128, 1024], F32, tag="psim2")
                                    for h0, hw in _halves(gw, wide):
                                        for kb in range(2):
                                            nc.tensor.matmul(
                                                psim[:rows, h0:h0 + hw],
                                                x0s[n][kb][:, l0:l0 + rows],
                                                x1s[n][kb][:, c0 + h0:c0 + h0 + hw],
                                                start=(kb == 0), stop=False)
                                        nc.tensor.matmul(
                                            psim[:rows, h0:h0 + hw],
                                            neg_ones[:, :rows],
                                            aug[:, cidx * 1024 + h0:
                                                cidx * 1024 + h0 + hw],
                                            start=False, stop=True)
                                    cchunk = cp.tile([128, 1024], BF16, tag="cchunk")
                                    nc.scalar.activation(
                                        cchunk[:rows, :gw],
                                        psim[:rows, :gw], AF.Exp,
                                        scale=1.0,
                                        bias=neg_lr[:rows, sidx:sidx + 1],
                                        accum_out=confsum_parts[:rows,
                                                                sidx * NGRP + g:
                                                                sidx * NGRP + g + 1])
                                    if do_dma:
                                        nc.sync.dma_start(
                                            conf_d[n, l0:l0 + rows, c0:c0 + gw],
                                            cchunk[:rows, :gw])

                if do_p2 and mode in ("exp2", "exp3"):
                    # P2: u = exp(2*sim - log rowsum - 6 ln2) on ACT (2048-
                    # wide), then conf = u * (2^6/colsum) as a 2x-mode DVE
                    # tensor_tensor against the broadcast fp16 colscale.
                    if p2w == 2048:
                        G2 = [(0, 2048), (2048, 2048), (4096, 2048),
                              (6144, 256)]
                    else:
                        G2 = list(GROUPS)
                    with tc.tile_pool(name="ps2", bufs=4096 // p2w,
                                      space="PSUM") as ps2:
                        for n in range(N):
                            for i, (l0, rows) in enumerate(STRIPS):
                                sidx = n * len(STRIPS) + i
                                for c0, gw in G2:
                                    psim = ps2.tile([128, p2w], F32, tag="psim2")
                                    step = 1024 if wide else 512
                                    for h0 in range(0, gw, step):
                                        hw = min(step, gw - h0)
                                        for kb in range(2):
                                            nc.tensor.matmul(
                                                psim[:rows, h0:h0 + hw],
                                                x0s[n][kb][:, l0:l0 + rows],
                                                x1s[n][kb][:, c0 + h0:c0 + h0 + hw],
                                                start=(kb == 0), stop=(kb == 1))
                                    u = tp.tile([128, p2w], BF16, tag="u")
                                    nc.scalar.activation(
                                        u[:rows, :gw], psim[:rows, :gw], AF.Exp,
                                        scale=1.0,
                                        bias=neg_lr[:rows, sidx:sidx + 1])
                                    cchunk = cp.tile([128, p2w], BF16,
                                                     tag="cchunk")
                                    base = (n * NGRP + c0 // 1024) * 1024
                                    nc.vector.tensor_tensor(
                                        cchunk[:rows, :gw], u[:rows, :gw],
                                        cs_b[:rows, base:base + gw], op=MUL)
                                    if do_dma:
                                        nc.sync.dma_start(
                                            conf_d[n, l0:l0 + rows, c0:c0 + gw],
                                            cchunk[:rows, :gw])

                if do_p2 and mode == "dve":
                    for n in range(N):
                        for i, (l0, rows) in enumerate(STRIPS):
                            sidx = n * len(STRIPS) + i
                            for g, (c0, gw) in enumerate(GROUPS):
                                cidx = n * NGRP + g
                                el = lp.tile([128, 1024], FP16, tag="el")
                                nc.gpsimd.dma_start(
                                    el[:rows, :gw],
                                    E_d[n, l0:l0 + rows, c0:c0 + gw])
                                tmp = tp.tile([128, 1024], FP16, tag="tmp")
                                # tmp = (E * 2^12/rowsum[p]) * E
                                nc.vector.scalar_tensor_tensor(
                                    tmp[:rows, :gw], el[:rows, :gw],
                                    rs_t[:rows, sidx:sidx + 1], el[:rows, :gw],
                                    op0=MUL, op1=MUL)
                                cchunk = cp.tile([128, 1024], BF16, tag="cchunk")
                                # conf = (tmp * 2^-18) * (2^6/colsum[f]);
                                # accum = sum_f conf
                                nc.vector.scalar_tensor_tensor(
                                    cchunk[:rows, :gw], tmp[:rows, :gw],
                                    float(2.0 ** -18),
                                    cs_b[:rows, cidx * 1024:cidx * 1024 + gw],
                                    op0=MUL, op1=MUL,
                                    accum_out=confsum_parts[:rows,
                                                            sidx * NGRP + g:
                                                            sidx * NGRP + g + 1])
                                if do_dma:
                                    nc.sync.dma_start(
                                        conf_d[n, l0:l0 + rows, c0:c0 + gw],
                                        cchunk[:rows, :gw])

                if do_p2:
                    nc.vector.reduce_sum(
                        confsum_tot[:],
                        confsum_parts[:].rearrange("p (s j) -> p s j", j=NGRP),
                        axis=mybir.AxisListType.X)
            nc.sync.dma_start(rowstat_d[:, :], confsum_tot[:])

    return nc


_cache = {}


def _get_kernel(reps=1, do_p1=True, do_p2=True, do_dma=True, mode="exp2",
                no_ar=False, p2w=2048):
    key = (reps, do_p1, do_p2, do_dma, mode, no_ar, p2w)
    if key not in _cache:
        _install_bir_fix()
        _cache[key] = build_kernel(reps, do_p1=do_p1, do_p2=do_p2,
                                   do_dma=do_dma, mode=mode, no_ar=no_ar,
                                   p2w=p2w)
    return _cache[key]


# ---------------------------------------------------------------------------
# Cached PJRT runner: jit(shard_map(bass_exec)) over the 8 axon NeuronCores.
# Mirrors concourse.bass2jax.run_bass_via_pjrt but (a) caches the jitted
# callable, (b) keeps inputs + output placeholders device-resident, and
# (c) does NOT donate the placeholders, so repeat invocations transfer no
# host data at all (the kernel writes every element of every output).
# ---------------------------------------------------------------------------
class _Runner:
    def __init__(self, nc):
        import jax
        import concourse.mybir as mybir
        from concourse import bass2jax

        bass2jax.install_neuronx_cc_hook()
        self._jax = jax
        self._b2j = bass2jax

        in_names, out_names, out_avals, out_np = [], [], [], []
        partition_name = (nc.partition_id_tensor.name
                          if nc.partition_id_tensor else None)
        for alloc in nc.m.functions[0].allocations:
            if not isinstance(alloc, mybir.MemoryLocationSet):
                continue
            name = alloc.memorylocations[0].name
            if alloc.kind == "ExternalInput":
                if name != partition_name:
                    in_names.append(name)
            elif alloc.kind == "ExternalOutput":
                shape = tuple(alloc.tensor_shape)
                dtype = mybir.dt.np(alloc.dtype)
                out_names.append(name)
                out_avals.append(jax.core.ShapedArray(shape, dtype))
                out_np.append((shape, dtype))
        self.in_params = list(in_names)
        self.out_names = list(out_names)
        self.out_np = out_np
        n_params = len(in_names)
        in_names = in_names + out_names   # placeholders ride as extra inputs
        if partition_name is not None:
            in_names = in_names + [partition_name]

        def _body(*args):
            operands = list(args)
            if partition_name is not None:
                operands.append(bass2jax.partition_id_tensor())
            outs = bass2jax._bass_exec_p.bind(
                *operands,
                out_avals=tuple(out_avals),
                in_names=tuple(in_names),
                out_names=tuple(out_names),
                lowering_input_output_aliases=(),
                sim_require_finite=True,
                sim_require_nnan=True,
                nc=nc,
            )
            return tuple(outs)

        from jax.sharding import Mesh, NamedSharding, PartitionSpec
        from jax.experimental.shard_map import shard_map

        devices = jax.devices("axon")[:NCORES]
        assert len(devices) == NCORES
        self.mesh = Mesh(np.asarray(devices), ("core",))
        self.sharding = NamedSharding(self.mesh, PartitionSpec("core"))
        n_outs = len(out_names)
        in_specs = (PartitionSpec("core"),) * (n_params + n_outs)
        out_specs = (PartitionSpec("core"),) * n_outs
        self.fn = jax.jit(
            shard_map(_body, mesh=self.mesh, in_specs=in_specs,
                      out_specs=out_specs, check_rep=False),
            keep_unused=True)
        self._placeholders = None
        self._dev_inputs = None
        self._dev_inputs_key = None

    def _get_placeholders(self):
        if self._placeholders is None:
            import jax.numpy as jnp
            jax = self._jax
            mk = jax.jit(
                lambda: tuple(
                    jnp.zeros((NCORES * s[0], *s[1:]), d)
                    for s, d in self.out_np),
                out_shardings=tuple(self.sharding for _ in self.out_np))
            self._placeholders = mk()
        return self._placeholders

    def put_inputs(self, in_map, key=None):
        """in_map: name -> per-core-stacked array [NCORES*shape0, ...]."""
        jax = self._jax
        if key is not None and key == self._dev_inputs_key:
            return
        self._dev_inputs = [
            jax.device_put(in_map[name], self.sharding)
            for name in self.in_params]
        for a in self._dev_inputs:
            a.block_until_ready()
        self._dev_inputs_key = key

    def run(self, fetch=True):
        outs = self.fn(*self._dev_inputs, *self._get_placeholders())
        if not fetch:
            return outs
        res = {}
        for name, arr, (shape, _) in zip(self.out_names, outs, self.out_np):
            res[name] = np.asarray(arr).reshape(NCORES, *shape)
        return res


_runners = {}


def _get_runner(reps=1, mode="exp2", **kw):
    key = (reps, mode) + tuple(sorted(kw.items()))
    if key not in _runners:
        _runners[key] = _Runner(_get_kernel(reps, mode=mode, **kw))
    return _runners[key]


def bench_runners(specs, inputs, trials=8):
    """specs: list of (label, runner-kwargs). Shares device inputs across
    runners; returns {label: sorted wall times}."""
    import time as _t
    shared = {}
    out = {}
    for label, kw in specs:
        t0 = _t.time()
        for attempt in range(3):
            try:
                r = _get_runner(**kw)
                if "inp" not in shared:
                    r.put_inputs(inputs, key="main")
                    shared["inp"] = r._dev_inputs
                    shared["ph"] = r._get_placeholders()
                else:
                    r._dev_inputs = shared["inp"]
                    r._placeholders = shared["ph"]
                o = r.run(fetch=False)
                [a.block_until_ready() for a in o]
                out[label] = r
                break
            except Exception as e:
                print(f"[{label}] attempt {attempt} failed: "
                      f"{type(e).__name__} {str(e)[:120]}", flush=True)
                _runners.pop((kw.get("reps", 1), kw.get("mode", "dve")) +
                             tuple(sorted({k: v for k, v in kw.items()
                                           if k not in ("reps", "mode")}
                                          .items())), None)
        else:
            print(f"[{label}] SKIPPED after retries", flush=True)
            continue
        print(f"[{label}] build+first: {_t.time()-t0:.1f}s", flush=True)
    walls = {}
    for label, _ in specs:
        if label not in out:
            continue
        r = out[label]
        w = []
        for _i in range(trials):
            t0 = _t.time()
            os_ = r.run(fetch=False)
            for a in os_:
                a.block_until_ready()
            w.append(_t.time() - t0)
        walls[label] = sorted(w)
    return out, walls


def _border_valid_np():
    def grid_valid(h, w):
        ih = np.arange(h)
        iw = np.arange(w)
        vh = (ih >= BORDER) & (ih < h - BORDER)
        vw = (iw >= BORDER) & (iw < w - BORDER)
        return (vh[:, None] & vw[None, :]).reshape(-1)
    v0 = grid_valid(H0, W0)
    v1 = grid_valid(H1, W1)
    return v0[:, None] & v1[None, :]


def _prep_inputs(x0, x1):
    import ml_dtypes
    bf16 = ml_dtypes.bfloat16
    x0t = np.ascontiguousarray(
        (np.asarray(x0, np.float32) * np.float32(SCALE2)).transpose(0, 2, 1)
    ).astype(bf16)                                     # [N, C, L]
    x1t = np.ascontiguousarray(
        np.asarray(x1, np.float32).transpose(0, 2, 1)).astype(bf16)
    # per-core stacked along axis 0: x0t shards, x1t replicated
    x0t_stack = np.concatenate(
        [x0t[:, :, k * LSH:(k + 1) * LSH] for k in range(NCORES)], axis=0)
    x1t_stack = np.concatenate([x1t] * NCORES, axis=0)
    return {"x0t": x0t_stack, "x1t": x1t_stack}


def run_device(x0, x1, reps=1, fetch=True, inputs_key=None, mode="exp2"):
    """Run the SPMD kernel; returns dict of stacked outputs (if fetch)."""
    r = _get_runner(reps, mode=mode)
    r.put_inputs(_prep_inputs(x0, x1), key=inputs_key)
    return r.run(fetch=fetch)


def kernel(x0, x1):
    res = run_device(x0, x1, reps=1, fetch=True)
    conf = np.concatenate(list(res["conf"]), axis=1).astype(np.float32)

    mask = np.zeros((N, L, S), dtype=bool)
    scores = np.zeros((N, L, S), dtype=np.float32)

    # If no confidence comes near THRESHOLD, mask/scores are exactly all-
    # zero (mask = conf > THRESHOLD & ...); the scan of our conf is exact.
    if float(conf.max()) > THRESHOLD * 0.95:
        # Exact reference semantics on our conf (never triggered for randn
        # inputs; kept for full generality).
        valid = _border_valid_np()[None]
        m = (conf > THRESHOLD) & valid
        m &= conf == conf.max(axis=2, keepdims=True)
        m &= conf == conf.max(axis=1, keepdims=True)
        mask = m
        scores = np.where(mask, conf, np.float32(0.0))

    return conf, mask, scores
